# revision 24
# baseline (speedup 1.0000x reference)
"""Trainium2 Bass kernel for the CRAFT-style hard-negative-mining MSE loss.

Reference math (per branch, over N = 16*768*768 flat pixels):
    all_loss = (pred - target)^2
    pos_mask = (target >= 0.3) & (weight != 0)
    neg_mask = (target < 0.1)
    pos_sum  = sum(pos_mask * all_loss * weight)
    k        = min(max(1000, 3*num_pos), num_neg)
    topk_sum = sum of k largest all_loss among negatives
    loss     = (pos_sum + topk_sum) / (num_pos + k)
    out      = loss_char + loss_aff

With uniform targets num_pos ~ 0.7*N, so 3*num_pos >> num_neg and
k == num_neg: the top-k degenerates to the full sum over negatives.

Device strategy (v8): per 1/8 shard, per branch-tile [128, W=2304]:
    DVE:  d  = p - t                    tensor_tensor        (2x mode)
          m_pos = (t >= 0.3)           tensor_scalar is_ge  (4x mode)
          mw = m_pos * w                tensor_tensor        (2x mode)
          m_neg = (t < 0.1)            tensor_scalar is_lt  (4x, mask
                                        tiles only)
    ACT:  l  = d^2                      Square               (1x)
          s_neg = sign(0.1 - t) (+-1)   Sign (1x, sign tiles only)
    PE:   24 blocks of 96 data cols:
        psum[0:97,0:385] += [l_96 | 1]^T @ [g0 | g1 | m_pos | mw | one]
          accumulated over the whole branch into one [97, 385] PSUM
          region.

The neg-mask work ALTERNATES tile-by-tile between ScalarE (as a +-1
Sign into group 0) and VectorE (as a 0/1 compare into group 1), so
that DVE, ScalarE, TensorE and DMA all sit just under the ~34us DMA
roofline instead of any one engine being the bottleneck.  The routing
is done by parity of two persistent moving buffers: the sign buffer
has zeros in group 1 and 1.0 in the ones-column (so column 384
accumulates sum(l) over sign tiles, needed to recover S1 from the +-1
encoding); the mask buffer has zeros in group 0 and 0.0 in the
ones-column.

PSUM contents:
    diag rows 0:96, group 0 -> <s_neg, l> = 2*S1_sign - sum_l_sign
    diag rows 0:96, group 1 -> <m_neg, l> = S1_mask
    diag rows 0:96, group 3 -> <mw,    l> = S2
    row 96, group 0         -> sum(s_neg) = 2*n_neg_sign - N/2
    row 96, group 1         -> n_neg_mask
    row 96, group 2         -> n_pos
    col 384 rows 0:96       -> sum(l) over sign tiles
The [97, 385] PSUM regions are copied to SBUF (ScalarE) and DMA'd out
in 4 partition slices over multiple DMA queues; the host recovers
S1/S2/counts, sums across the 8 shards, and applies the k/denominator
logic (with a full numpy fallback for the never-hit-here k < num_neg
case).  Inputs are cast to bf16 on the host: halves HBM traffic and
doubles DVE tensor_tensor throughput.
"""

import os
import numpy as np
import ml_dtypes

N_CORES = 8
B, H, W = 16, 768, 768
NPX = B * H * W              # 9_437_184 flat pixels
P = 128                      # SBUF partitions
FD = NPX // (N_CORES * P)    # 9216 free-dim elements per core per tensor
WIDTHS = [1152, 2304, 2304, 2304, 1152]   # tile widths per branch
N_TILES = len(WIDTHS)
TW = 2304                    # max tile width
BD = 96                      # data columns per matmul block
PR = BD + 1                  # psum rows used (96 data + 1 count row)
NB = TW // BD                # max matmul blocks per tile
MC = 4 * BD + 2              # moving row: g0 | g1 | m_pos | mw | one | pad
MR = MC - 1                  # columns actually read by the matmul (385)

THRESH_NEG = 0.1
THRESH_POS = 0.3

_compiled = None             # cached nc
LAST_RESULTS = None          # BassKernelResults of the last run (for profiling)


def _build_nc():
    import concourse.bacc as bacc
    import concourse.mybir as mybir
    import concourse.tile as tile
    from contextlib import ExitStack

    DT = mybir.dt.bfloat16
    f32 = mybir.dt.float32
    Alu = mybir.AluOpType
    Act = mybir.ActivationFunctionType

    nc = bacc.Bacc(
        "TRN2",
        target_bir_lowering=False,
        debug=False,
        num_devices=N_CORES,
    )

    # packed input: [P, branch, (p,t,w), FD]
    pk = nc.declare_dram_parameter("pk", [P, 2, 3, FD], DT, isOutput=False)
    out_ps = nc.declare_dram_parameter("acc_ps", [PR, 2, MR], f32, isOutput=True)

    with tile.TileContext(nc) as tc, ExitStack() as ctx:
        in_pool = ctx.enter_context(tc.tile_pool(name="in", bufs=3))
        d_pool = ctx.enter_context(tc.tile_pool(name="d", bufs=2))
        acc_pool = ctx.enter_context(tc.tile_pool(name="acc", bufs=1))
        ps_pool = ctx.enter_context(tc.psum_pool(name="ps", bufs=1))

        ps_sb = acc_pool.tile([PR, 2, MR], f32, tag="ps_sb")
        psum = [
            ps_pool.tile([PR, MR], f32, tag=f"psum{b}", name=f"psum{b}")
            for b in range(2)
        ]
        # per-partition f32 bias constant 0.1 for sign(0.1 - t)
        bias_neg = acc_pool.tile([P, 1], f32, tag="bias_neg")
        nc.gpsimd.memset(bias_neg[:], THRESH_NEG)
        # persistent double-buffered stationary [l_96 | 1] blocks; ones
        # column written once
        lexts = [
            acc_pool.tile([P, NB, PR], DT, tag=f"lext{j}", name=f"lext{j}")
            for j in range(2)
        ]
        # persistent moving buffers: ms[0] for sign tiles, ms[1] for mask
        # tiles; the unused mask group and the ones column are fixed once
        ms = [
            acc_pool.tile([P, NB, MC], DT, tag=f"m{j}", name=f"m{j}")
            for j in range(2)
        ]
        for j in range(2):
            nc.gpsimd.memset(lexts[j][:, :, BD : BD + 1], 1.0)
        nc.gpsimd.memset(ms[0][:, :, BD : 2 * BD], 0.0)          # group 1
        nc.gpsimd.memset(ms[0][:, :, 4 * BD : 4 * BD + 1], 1.0)  # ones col
        nc.gpsimd.memset(ms[1][:, :, 0:BD], 0.0)                 # group 0
        nc.gpsimd.memset(ms[1][:, :, 4 * BD : 4 * BD + 1], 0.0)  # ones col

        # PE warm-up: the HAM clock gate keeps TensorE at 1.2 GHz until it
        # has been busy ~3.4us.  The first real matmul only lands ~15us in
        # (after DMA+DVE+ACT of tile 0), so burn the idle head on dummy
        # matmuls into a scratch PSUM bank to reach 2.4 GHz before the
        # real stream starts.
        ps_warm = ps_pool.tile([P, 512], f32, tag="ps_warm")
        nc.gpsimd.memset(ms[1][:, 0, 0:MC], 0.0)
        for _ in range(34):
            nc.tensor.matmul(
                ps_warm[:, 0:MR],
                ms[1][:, 0, 0:128],
                ms[1][:, 0, 0:MR],
                start=True,
                stop=True,
            )

        it = 0
        for b in range(2):
            c0 = 0
            for i, Wt in enumerate(WIDTHS):
                nb = Wt // BD
                sign_tile = it % 2 == 0
                tin = in_pool.tile([P, 3, TW], DT, tag="in")
                nc.sync.dma_start(tin[:, :, 0:Wt], pk[:, b, :, c0 : c0 + Wt])
                pt = tin[:, 0, 0:Wt]
                tt = tin[:, 1, 0:Wt]
                wt = tin[:, 2, 0:Wt]

                lext = lexts[it % 2]
                m = ms[it % 2]
                if sign_tile:
                    # s_neg = sign(0.1 - t), +-1 exactly    (ACT Sign 1x)
                    nc.scalar.activation(
                        m[:, 0:nb, 0:BD], tt, Act.Sign,
                        bias=bias_neg[:], scale=-1.0,
                    )
                # d = pred - target                          (DVE TT 2x)
                d = d_pool.tile([P, TW], DT, tag="d")
                nc.vector.tensor_tensor(d[:, 0:Wt], pt, tt, Alu.subtract)
                # l = d^2 into cols 0:96 of the 97-blocks    (ACT Square 1x)
                nc.scalar.activation(
                    lext[:, 0:nb, 0:BD], d[:, 0:Wt], Act.Square
                )
                if not sign_tile:
                    # m_neg = (t < 0.1) into group 1         (DVE TS 4x)
                    nc.vector.tensor_scalar(
                        m[:, 0:nb, BD : 2 * BD], tt, THRESH_NEG, None,
                        Alu.is_lt,
                    )
                # m_pos = (t >= 0.3)                         (DVE TS 4x)
                nc.vector.tensor_scalar(
                    m[:, 0:nb, 2 * BD : 3 * BD], tt, THRESH_POS, None,
                    Alu.is_ge,
                )
                # mw = m_pos * w                             (DVE TT 2x)
                nc.vector.tensor_tensor(
                    m[:, 0:nb, 3 * BD : 4 * BD],
                    m[:, 0:nb, 2 * BD : 3 * BD],
                    wt,
                    Alu.mult,
                )

                # psum[b] += [l_blk | 1]^T @ [g0|g1|m_pos|mw|1]       (PE)
                for k in range(nb):
                    nc.tensor.matmul(
                        psum[b][:, :],
                        lext[:, k, :],
                        m[:, k, 0:MR],
                        start=(i == 0 and k == 0),
                        stop=(i == N_TILES - 1 and k == nb - 1),
                    )
                c0 += Wt
                it += 1

            # dump the accumulated [97, 385] PSUM region to SBUF (ScalarE),
            # then DMA it out in 4 partition slices on multiple engine
            # queues (a single contiguous store serializes on one DMA ring)
            nc.scalar.copy(ps_sb[:, b], psum[b][:, :])
            slices = [(0, 25), (25, 49), (49, 73), (73, PR)]
            issuers = [nc.sync, nc.gpsimd, nc.scalar, nc.gpsimd]
            for (p0, p1), eng in zip(slices, issuers):
                eng.dma_start(out_ps[p0:p1, b], ps_sb[p0:p1, b])

    nc.compile()
    return nc


def _get_nc():
    global _compiled
    if _compiled is None:
        _compiled = _build_nc()
    return _compiled


def _np_branch_fallback(pred, target, weight):
    """Exact reference math in numpy float64 (handles k < num_neg)."""
    pred = pred.astype(np.float64)
    target = target.astype(np.float64)
    weight = weight.astype(np.float64)
    all_loss = (pred - target) ** 2
    pos_mask = (target >= THRESH_POS) & (weight != 0)
    neg_mask = target < THRESH_NEG
    pos_sum = float(np.sum(np.where(pos_mask, all_loss * weight, 0.0)))
    num_pos = int(np.sum(pos_mask))
    num_neg = int(np.sum(neg_mask))
    k = min(max(1000, 3 * num_pos), num_neg)
    neg_vals = all_loss[neg_mask]
    if k >= num_neg:
        topk = float(neg_vals.sum())
    elif k <= 0:
        topk = 0.0
    else:
        topk = float(np.partition(neg_vals, num_neg - k)[num_neg - k :].sum())
    return (pos_sum + topk) / (num_pos + k)


def kernel(output, character_map, affinity_map, character_weight, affinity_weight):
    from concourse.bass_utils import run_bass_kernel_spmd

    global LAST_RESULTS
    np_dt = ml_dtypes.bfloat16

    output = np.asarray(output, dtype=np.float32)

    def shard(a):
        # flat pixel order (b, h, w) -> [core, partition, free]
        return np.ascontiguousarray(a).reshape(N_CORES, P, FD).astype(np_dt)

    packed = np.empty((N_CORES, P, 2, 3, FD), dtype=np_dt)
    packed[:, :, 0, 0] = shard(output[:, 0])
    packed[:, :, 0, 1] = shard(np.asarray(character_map, dtype=np.float32))
    packed[:, :, 0, 2] = shard(np.asarray(character_weight, dtype=np.float32))
    packed[:, :, 1, 0] = shard(output[:, 1])
    packed[:, :, 1, 1] = shard(np.asarray(affinity_map, dtype=np.float32))
    packed[:, :, 1, 2] = shard(np.asarray(affinity_weight, dtype=np.float32))

    in_maps = [{"pk": packed[c]} for c in range(N_CORES)]

    nc = _get_nc()
    res = run_bass_kernel_spmd(
        nc,
        in_maps,
        list(range(N_CORES)),
        trace=os.environ.get("KERNEL_TRACE", "0") == "1",
    )
    LAST_RESULTS = res

    # [cores, PR, branch, col], col: [g0 0:96 | g1 96:192 | m_pos 192:288 |
    #                                 mw 288:384 | ones 384]
    acc_ps = np.stack([r["acc_ps"] for r in res.results]).astype(np.float64)
    idx = np.arange(BD)
    d0 = acc_ps[:, idx, :, idx].sum(axis=(0, 1))               # <s_neg, l>
    d1 = acc_ps[:, idx, :, BD + idx].sum(axis=(0, 1))          # S1_mask
    s2 = acc_ps[:, idx, :, 3 * BD + idx].sum(axis=(0, 1))      # <mw, l>
    sum_l_s = acc_ps[:, 0:BD, :, 4 * BD].sum(axis=(0, 1))      # sum_l sign
    r0 = acc_ps[:, BD, :, 0:BD].sum(axis=(0, 2))               # sum(s_neg)
    r1 = acc_ps[:, BD, :, BD : 2 * BD].sum(axis=(0, 2))        # n_neg_mask
    n_pos = acc_ps[:, BD, :, 2 * BD : 3 * BD].sum(axis=(0, 2))  # num_pos

    # per branch, sign tiles cover exactly half the branch's pixels
    n_sign = NPX / 2.0
    s1 = (d0 + sum_l_s) / 2.0 + d1
    n_neg = (r0 + n_sign) / 2.0 + r1

    total = 0.0
    for bidx, (tmap, wmap) in enumerate(
        [(character_map, character_weight), (affinity_map, affinity_weight)]
    ):
        num_neg = int(round(n_neg[bidx]))
        num_pos = int(round(n_pos[bidx]))
        k = min(max(1000, 3 * num_pos), num_neg)
        if k == num_neg:
            total += (s1[bidx] + s2[bidx]) / (num_pos + k)
        else:
            # top-k actually selective: fall back to exact host computation
            total += _np_branch_fallback(
                output[:, bidx].reshape(-1),
                np.asarray(tmap, dtype=np.float32).reshape(-1),
                np.asarray(wmap, dtype=np.float32).reshape(-1),
            )

    return np.float32(total)


# revision 25
# speedup vs baseline: 1.0055x; 1.0055x over previous
"""Trainium2 Bass kernel for the CRAFT-style hard-negative-mining MSE loss.

Reference math (per branch, over N = 16*768*768 flat pixels):
    all_loss = (pred - target)^2
    pos_mask = (target >= 0.3) & (weight != 0)
    neg_mask = (target < 0.1)
    pos_sum  = sum(pos_mask * all_loss * weight)
    k        = min(max(1000, 3*num_pos), num_neg)
    topk_sum = sum of k largest all_loss among negatives
    loss     = (pos_sum + topk_sum) / (num_pos + k)
    out      = loss_char + loss_aff

With uniform targets num_pos ~ 0.7*N, so 3*num_pos >> num_neg and
k == num_neg: the top-k degenerates to the full sum over negatives.

Device strategy (v11): per 1/8 shard, per branch-tile [128, W]:
    DVE:  d  = p - t                    tensor_tensor        (2x mode)
          m_pos = (t >= 0.3)           tensor_scalar is_ge  (4x mode)
          mw = m_pos * w                tensor_tensor        (2x mode)
          g0 = 2*(t < 0.1)             tensor_scalar is_lt,
                                        *2 (4x, mask tiles only)
    ACT:  l  = d^2                      Square               (1x)
          g0 = sign(0.1 - t) (+-1)      Sign (1x, sign tiles only)
    PE:   blocks of 96 data cols:
        psum[0:97, 0:289] += [l_96 | 1]^T @ [g0 | m_pos | mw | one]
          accumulated over the whole branch into one [97, 289] PSUM
          region.

The neg-mask work ALTERNATES tile-by-tile between ScalarE (+-1 Sign)
and VectorE (a 0/2 compare), so DVE and ScalarE share the load and
both sit under the ~34us DMA roofline.  Both encodings share PSUM
group 0: sign tiles contribute <s_neg,l> = 2*S1_t - sum(l_t), mask
tiles contribute <2*m_neg,l> = 2*S1_t, so with the ones-column of the
sign buffer set to 1.0 (and the mask buffer's to 0.0), column 288
accumulates exactly sum(l) over sign tiles and

    S1    = (diag_g0 + col288) / 2
    n_neg = (row96_g0 + N/2) / 2     (sign tiles cover half the pixels)
    n_pos = row96_g1,  S2 = diag_g2

The [97, 289] PSUM regions are copied to SBUF (ScalarE) and DMA'd out
in 4 partition slices over multiple DMA queues; the host applies the
k/denominator logic (with a full numpy fallback for the never-hit-here
k < num_neg case).

TensorE is pre-warmed with dummy matmuls during the NEFF preamble so
the HAM clock gate reaches 2.4 GHz before the first real matmul.
Inputs are cast to bf16 on the host: halves HBM traffic and doubles
DVE tensor_tensor throughput.  Tile widths are smaller at the pipeline
ends (fill/drain latency) and wide in the DMA-saturated middle.
"""

import os
import numpy as np
import ml_dtypes

N_CORES = 8
B, H, W = 16, 768, 768
NPX = B * H * W              # 9_437_184 flat pixels
P = 128                      # SBUF partitions
FD = NPX // (N_CORES * P)    # 9216 free-dim elements per core per tensor
WIDTHS = [1152, 2304, 2304, 2304, 1152]   # tile widths per branch
N_TILES = len(WIDTHS)
TW = 2304                    # max tile width
BD = 96                      # data columns per matmul block
PR = BD + 1                  # psum rows used (96 data + 1 count row)
NB = TW // BD                # max matmul blocks per tile
MC = 3 * BD + 2              # moving row: g0 | m_pos | mw | one | pad
MR = MC - 1                  # columns actually read by the matmul (289)

THRESH_NEG = 0.1
THRESH_POS = 0.3

_compiled = None             # cached nc
LAST_RESULTS = None          # BassKernelResults of the last run (for profiling)


def _build_nc():
    import concourse.bacc as bacc
    import concourse.mybir as mybir
    import concourse.tile as tile
    from contextlib import ExitStack

    DT = mybir.dt.bfloat16
    f32 = mybir.dt.float32
    Alu = mybir.AluOpType
    Act = mybir.ActivationFunctionType

    nc = bacc.Bacc(
        "TRN2",
        target_bir_lowering=False,
        debug=False,
        num_devices=N_CORES,
    )

    # packed input: [P, branch, (p,t,w), FD]
    pk = nc.declare_dram_parameter("pk", [P, 2, 3, FD], DT, isOutput=False)
    out_ps = nc.declare_dram_parameter("acc_ps", [PR, 2, MR], f32, isOutput=True)

    with tile.TileContext(nc) as tc, ExitStack() as ctx:
        in_pool = ctx.enter_context(tc.tile_pool(name="in", bufs=3))
        d_pool = ctx.enter_context(tc.tile_pool(name="d", bufs=2))
        acc_pool = ctx.enter_context(tc.tile_pool(name="acc", bufs=1))
        ps_pool = ctx.enter_context(tc.psum_pool(name="ps", bufs=1))

        ps_sb = acc_pool.tile([PR, 2, MR], f32, tag="ps_sb")
        psum = [
            ps_pool.tile([PR, MR], f32, tag=f"psum{b}", name=f"psum{b}")
            for b in range(2)
        ]
        # per-partition f32 bias constant 0.1 for sign(0.1 - t)
        bias_neg = acc_pool.tile([P, 1], f32, tag="bias_neg")
        nc.gpsimd.memset(bias_neg[:], THRESH_NEG)
        # persistent double-buffered stationary [l_96 | 1] blocks; ones
        # column written once
        lexts = [
            acc_pool.tile([P, NB, PR], DT, tag=f"lext{j}", name=f"lext{j}")
            for j in range(2)
        ]
        # persistent moving buffers: ms[0] for sign tiles (ones col 1.0),
        # ms[1] for mask tiles (ones col 0.0)
        ms = [
            acc_pool.tile([P, NB, MC], DT, tag=f"m{j}", name=f"m{j}")
            for j in range(2)
        ]
        for j in range(2):
            nc.gpsimd.memset(lexts[j][:, :, BD : BD + 1], 1.0)
            nc.gpsimd.memset(ms[j][:, :, 3 * BD : 3 * BD + 1], float(1 - j))

        # PE warm-up: the HAM clock gate keeps TensorE at 1.2 GHz until it
        # has been busy ~3.4us.  The first real matmul only lands ~15us in
        # (after DMA+DVE+ACT of tile 0), so burn the idle head on dummy
        # matmuls into a scratch PSUM bank to reach 2.4 GHz first.
        ps_warm = ps_pool.tile([P, 512], f32, tag="ps_warm")
        nc.gpsimd.memset(ms[1][:, 0, 0:MC], 0.0)
        for _ in range(34):
            nc.tensor.matmul(
                ps_warm[:, 0:MR],
                ms[1][:, 0, 0:128],
                ms[1][:, 0, 0:MR],
                start=True,
                stop=True,
            )

        it = 0
        for b in range(2):
            c0 = 0
            for i, Wt in enumerate(WIDTHS):
                nb = Wt // BD
                sign_tile = it % 2 == 0
                tin = in_pool.tile([P, 3, TW], DT, tag="in")
                nc.sync.dma_start(tin[:, :, 0:Wt], pk[:, b, :, c0 : c0 + Wt])
                pt = tin[:, 0, 0:Wt]
                tt = tin[:, 1, 0:Wt]
                wt = tin[:, 2, 0:Wt]

                lext = lexts[it % 2]
                m = ms[it % 2]
                # d = pred - target                          (DVE TT 2x)
                d = d_pool.tile([P, TW], DT, tag="d")
                nc.vector.tensor_tensor(d[:, 0:Wt], pt, tt, Alu.subtract)
                # l = d^2 into cols 0:96 of the 97-blocks    (ACT Square 1x)
                nc.scalar.activation(
                    lext[:, 0:nb, 0:BD], d[:, 0:Wt], Act.Square
                )
                if sign_tile:
                    # g0 = sign(0.1 - t), +-1 exactly        (ACT Sign 1x)
                    nc.scalar.activation(
                        m[:, 0:nb, 0:BD], tt, Act.Sign,
                        bias=bias_neg[:], scale=-1.0,
                    )
                else:
                    # g0 = 2*(t < 0.1), 0/2 exactly          (DVE TS 4x)
                    nc.vector.tensor_scalar(
                        m[:, 0:nb, 0:BD], tt, THRESH_NEG, 2.0,
                        Alu.is_lt, Alu.mult,
                    )
                # m_pos = (t >= 0.3)                         (DVE TS 4x)
                nc.vector.tensor_scalar(
                    m[:, 0:nb, BD : 2 * BD], tt, THRESH_POS, None, Alu.is_ge
                )
                # mw = m_pos * w                             (DVE TT 2x)
                nc.vector.tensor_tensor(
                    m[:, 0:nb, 2 * BD : 3 * BD],
                    m[:, 0:nb, BD : 2 * BD],
                    wt,
                    Alu.mult,
                )

                # psum[b] += [l_blk | 1]^T @ [g0 | m_pos | mw | 1]    (PE)
                for k in range(nb):
                    nc.tensor.matmul(
                        psum[b][:, :],
                        lext[:, k, :],
                        m[:, k, 0:MR],
                        start=(i == 0 and k == 0),
                        stop=(i == N_TILES - 1 and k == nb - 1),
                    )
                c0 += Wt
                it += 1

            # dump the accumulated [97, 289] PSUM region to SBUF (ScalarE),
            # then DMA it out in 4 partition slices on multiple engine
            # queues (a single contiguous store serializes on one DMA ring)
            nc.scalar.copy(ps_sb[:, b], psum[b][:, :])
            slices = [(0, 25), (25, 49), (49, 73), (73, PR)]
            issuers = [nc.sync, nc.gpsimd, nc.scalar, nc.gpsimd]
            for (p0, p1), eng in zip(slices, issuers):
                eng.dma_start(out_ps[p0:p1, b], ps_sb[p0:p1, b])

    nc.compile()
    return nc


def _get_nc():
    global _compiled
    if _compiled is None:
        _compiled = _build_nc()
    return _compiled


def _np_branch_fallback(pred, target, weight):
    """Exact reference math in numpy float64 (handles k < num_neg)."""
    pred = pred.astype(np.float64)
    target = target.astype(np.float64)
    weight = weight.astype(np.float64)
    all_loss = (pred - target) ** 2
    pos_mask = (target >= THRESH_POS) & (weight != 0)
    neg_mask = target < THRESH_NEG
    pos_sum = float(np.sum(np.where(pos_mask, all_loss * weight, 0.0)))
    num_pos = int(np.sum(pos_mask))
    num_neg = int(np.sum(neg_mask))
    k = min(max(1000, 3 * num_pos), num_neg)
    neg_vals = all_loss[neg_mask]
    if k >= num_neg:
        topk = float(neg_vals.sum())
    elif k <= 0:
        topk = 0.0
    else:
        topk = float(np.partition(neg_vals, num_neg - k)[num_neg - k :].sum())
    return (pos_sum + topk) / (num_pos + k)


def kernel(output, character_map, affinity_map, character_weight, affinity_weight):
    from concourse.bass_utils import run_bass_kernel_spmd

    global LAST_RESULTS
    np_dt = ml_dtypes.bfloat16

    output = np.asarray(output, dtype=np.float32)

    def shard(a):
        # flat pixel order (b, h, w) -> [core, partition, free]
        return np.ascontiguousarray(a).reshape(N_CORES, P, FD).astype(np_dt)

    packed = np.empty((N_CORES, P, 2, 3, FD), dtype=np_dt)
    packed[:, :, 0, 0] = shard(output[:, 0])
    packed[:, :, 0, 1] = shard(np.asarray(character_map, dtype=np.float32))
    packed[:, :, 0, 2] = shard(np.asarray(character_weight, dtype=np.float32))
    packed[:, :, 1, 0] = shard(output[:, 1])
    packed[:, :, 1, 1] = shard(np.asarray(affinity_map, dtype=np.float32))
    packed[:, :, 1, 2] = shard(np.asarray(affinity_weight, dtype=np.float32))

    in_maps = [{"pk": packed[c]} for c in range(N_CORES)]

    nc = _get_nc()
    res = run_bass_kernel_spmd(
        nc,
        in_maps,
        list(range(N_CORES)),
        trace=os.environ.get("KERNEL_TRACE", "0") == "1",
    )
    LAST_RESULTS = res

    # [cores, PR, branch, col], col: [g0 0:96 | m_pos 96:192 | mw 192:288 |
    #                                 ones 288]
    acc_ps = np.stack([r["acc_ps"] for r in res.results]).astype(np.float64)
    idx = np.arange(BD)
    d0 = acc_ps[:, idx, :, idx].sum(axis=(0, 1))               # mixed g0
    s2 = acc_ps[:, idx, :, 2 * BD + idx].sum(axis=(0, 1))      # <mw, l>
    sum_l_s = acc_ps[:, 0:BD, :, 3 * BD].sum(axis=(0, 1))      # sum_l sign
    r0 = acc_ps[:, BD, :, 0:BD].sum(axis=(0, 2))               # mixed row
    n_pos = acc_ps[:, BD, :, BD : 2 * BD].sum(axis=(0, 2))     # num_pos

    # per branch, sign tiles cover exactly half the branch's pixels
    n_sign = NPX / 2.0
    s1 = (d0 + sum_l_s) / 2.0
    n_neg = (r0 + n_sign) / 2.0

    total = 0.0
    for bidx, (tmap, wmap) in enumerate(
        [(character_map, character_weight), (affinity_map, affinity_weight)]
    ):
        num_neg = int(round(n_neg[bidx]))
        num_pos = int(round(n_pos[bidx]))
        k = min(max(1000, 3 * num_pos), num_neg)
        if k == num_neg:
            total += (s1[bidx] + s2[bidx]) / (num_pos + k)
        else:
            # top-k actually selective: fall back to exact host computation
            total += _np_branch_fallback(
                output[:, bidx].reshape(-1),
                np.asarray(tmap, dtype=np.float32).reshape(-1),
                np.asarray(wmap, dtype=np.float32).reshape(-1),
            )

    return np.float32(total)


# revision 34
# speedup vs baseline: 1.1628x; 1.1564x over previous
"""Trainium2 Bass kernel for the CRAFT-style hard-negative-mining MSE loss.

Reference math (per branch, over N = 16*768*768 flat pixels):
    all_loss = (pred - target)^2
    pos_mask = (target >= 0.3) & (weight != 0)
    neg_mask = (target < 0.1)
    pos_sum  = sum(pos_mask * all_loss * weight)
    k        = min(max(1000, 3*num_pos), num_neg)
    topk_sum = sum of k largest all_loss among negatives
    loss     = (pos_sum + topk_sum) / (num_pos + k)
    out      = loss_char + loss_aff

With uniform targets num_pos ~ 0.7*N, so 3*num_pos >> num_neg and
k == num_neg: the top-k degenerates to the full sum over negatives.

Device strategy (v5, TensorE-assisted): per 1/8 shard, per branch-tile
[128, W]:
    DVE:  d  = p - t                    tensor_tensor        (2x mode)
          m_neg = (t < 0.1)            tensor_scalar is_lt  (4x mode)
          m_pos = (t >= 0.3)           tensor_scalar is_ge  (4x mode)
          mw = m_pos * w                tensor_tensor        (2x mode)
    ACT:  l  = d^2                      Square (1x) -> strided [nb, 97]
    PE:   nb = W/96 blocks of 96 data cols:
            psum[0:97, 0:288] += [l_96 | 1]^T @ [m_neg | m_pos | mw]
          accumulated over the whole branch into one [97, 288] PSUM
          region:
            diag(rows 0:96 of group 0) -> per-col <m_neg, l> = S1
            diag(rows 0:96 of group 2) -> per-col <mw,    l> = S2
            row 96 of group 0          -> per-col sums of m_neg = num_neg
            row 96 of group 1          -> per-col sums of m_pos = num_pos
The [97, 288] PSUM regions are copied to SBUF (ScalarE) and DMA'd out
in 4 partition slices over 4 DMA queues; the host extracts
diagonals/count-rows, sums across the 8 shards, and applies the
k/denominator logic (with a full numpy fallback for the never-hit-here
k < num_neg case).

Tile widths are uneven on purpose: DVE per-op fixed overhead (~0.3us
of init bubble + drain per instruction) favors wide ops, but wide ops
at the pipeline ends serialize against DMA.  So tiles are small at the
very start (quick pipeline fill) and very end (short drain tail), and
wide in the DMA-saturated middle.

This moves the masked-sum and count reductions off DVE/ACT (where they
only run at 1x) onto the otherwise-idle TensorE.  Inputs are cast to
bf16 on the host: halves HBM traffic and doubles DVE tensor_tensor
throughput.
"""

import os
import numpy as np
import ml_dtypes

N_CORES = 8
B, H, W = 16, 768, 768
NPX = B * H * W              # 9_437_184 flat pixels
P = 128                      # SBUF partitions
FD = NPX // (N_CORES * P)    # 9216 free-dim elements per core per tensor
BD = 96                      # data columns per matmul block
PR = BD + 1                  # psum rows used (96 data + 1 count row)
WMAX = 2304                  # widest tile

# per-branch tile widths: small tiles at the global start (pipeline
# fill) and global end (drain tail) only
WIDTHS = [
    [1152, 1152, 2304, 2304, 2304],   # branch 0
    [2304, 2304, 2304, 1152, 1152],   # branch 1
]

THRESH_NEG = 0.1
THRESH_POS = 0.3

_compiled = None             # cached nc
LAST_RESULTS = None          # BassKernelResults of the last run (for profiling)


def _build_nc():
    import concourse.bacc as bacc
    import concourse.mybir as mybir
    import concourse.tile as tile
    from contextlib import ExitStack

    DT = mybir.dt.bfloat16
    f32 = mybir.dt.float32
    Alu = mybir.AluOpType
    Act = mybir.ActivationFunctionType

    nc = bacc.Bacc(
        "TRN2",
        target_bir_lowering=False,
        debug=False,
        num_devices=N_CORES,
    )

    # packed input: [P, branch, (p,t,w), FD]
    pk = nc.declare_dram_parameter("pk", [P, 2, 3, FD], DT, isOutput=False)
    out_ps = nc.declare_dram_parameter("acc_ps", [PR, 2, 3, BD], f32, isOutput=True)

    with tile.TileContext(nc) as tc, ExitStack() as ctx:
        in_pool = ctx.enter_context(tc.tile_pool(name="in", bufs=4))
        d_pool = ctx.enter_context(tc.tile_pool(name="d", bufs=2))
        m_pool = ctx.enter_context(tc.tile_pool(name="m", bufs=3))
        acc_pool = ctx.enter_context(tc.tile_pool(name="acc", bufs=1))
        ps_pool = ctx.enter_context(tc.psum_pool(name="ps", bufs=1))

        ps_sb = acc_pool.tile([PR, 2, 3, BD], f32, tag="ps_sb")
        psum = [
            ps_pool.tile([PR, 3, BD], f32, tag=f"psum{b}", name=f"psum{b}")
            for b in range(2)
        ]
        # persistent double-buffered [l | ones] stationary tensors; the
        # ones column (col 96 of each 97-block) is written once up front
        # and never touched again
        NBMAX = WMAX // BD
        lexts = [
            acc_pool.tile([P, NBMAX, PR], DT, tag=f"lext{j}", name=f"lext{j}")
            for j in range(2)
        ]
        for j in range(2):
            nc.gpsimd.memset(lexts[j][:, 0:3, 0:PR], 0.0)
            nc.gpsimd.memset(lexts[j][:, :, BD : BD + 1], 1.0)

        # PE warm-up: the HAM clock gate keeps TensorE at 1.2 GHz until it
        # has been busy ~3.4us.  The first real matmul only lands ~15us in
        # (after DMA+DVE+ACT of tile 0), so burn the idle head on dummy
        # matmuls into a scratch PSUM bank so the PE is at 2.4 GHz when
        # the real stream starts.
        ps_warm = ps_pool.tile([PR, 3 * PR], f32, tag="ps_warm")
        for _ in range(34):
            nc.tensor.matmul(
                ps_warm[:, :],
                lexts[0][:, 0, :],
                lexts[0][:, 0:3, :],
                start=True,
                stop=True,
            )

        it = 0
        for b in range(2):
            c0 = 0
            for i, Wt in enumerate(WIDTHS[b]):
                nb = Wt // BD
                sl_in = slice(c0, c0 + Wt)
                tin = in_pool.tile([P, 3, WMAX], DT, tag="in")
                nc.sync.dma_start(tin[:, :, 0:Wt], pk[:, b, :, sl_in])
                pt = tin[:, 0, 0:Wt]
                tt = tin[:, 1, 0:Wt]
                wt = tin[:, 2, 0:Wt]

                # d = pred - target first, so ACT can start  (DVE TT 2x)
                d = d_pool.tile([P, WMAX], DT, tag="d")
                nc.vector.tensor_tensor(d[:, 0:Wt], pt, tt, Alu.subtract)
                # l = d^2 into cols 0:96 of the 97-blocks, overlapping the
                # mask ops below                             (ACT Square 1x)
                lext = lexts[it % 2]
                nc.scalar.activation(
                    lext[:, 0:nb, 0:BD], d[:, 0:Wt], Act.Square
                )
                # masks (DVE TS 4x):  m[:,0]=(t<0.1)  m[:,1]=(t>=0.3)
                m = m_pool.tile([P, 3, WMAX], DT, tag="m")
                nc.vector.tensor_scalar(
                    m[:, 0, 0:Wt], tt, THRESH_NEG, None, Alu.is_lt
                )
                nc.vector.tensor_scalar(
                    m[:, 1, 0:Wt], tt, THRESH_POS, None, Alu.is_ge
                )
                # m[:,2] = m_pos * w                         (DVE TT 2x)
                nc.vector.tensor_tensor(
                    m[:, 2, 0:Wt], m[:, 1, 0:Wt], wt, Alu.mult
                )

                # psum[b] += [l_blk | 1]^T @ [m_neg | m_pos | mw]   (PE)
                for k in range(nb):
                    sl = slice(k * BD, (k + 1) * BD)
                    nc.tensor.matmul(
                        psum[b][:, :, :],
                        lext[:, k, :],
                        m[:, :, sl],
                        start=(i == 0 and k == 0),
                        stop=(i == len(WIDTHS[b]) - 1 and k == nb - 1),
                    )
                c0 += Wt
                it += 1

            # dump the accumulated [97, 288] PSUM region to SBUF (ScalarE),
            # then DMA it out in 4 partition slices on 4 different engine
            # queues (a single contiguous store serializes on one DMA ring)
            nc.scalar.copy(ps_sb[:, b], psum[b][:, :, :])
            slices = [(0, 25), (25, 49), (49, 73), (73, PR)]
            issuers = [nc.sync, nc.gpsimd, nc.scalar, nc.gpsimd]
            for (p0, p1), eng in zip(slices, issuers):
                eng.dma_start(out_ps[p0:p1, b], ps_sb[p0:p1, b])

    nc.compile()
    return nc


def _get_nc():
    global _compiled
    if _compiled is None:
        _compiled = _build_nc()
    return _compiled


def _np_branch_fallback(pred, target, weight):
    """Exact reference math in numpy float64 (handles k < num_neg)."""
    pred = pred.astype(np.float64)
    target = target.astype(np.float64)
    weight = weight.astype(np.float64)
    all_loss = (pred - target) ** 2
    pos_mask = (target >= THRESH_POS) & (weight != 0)
    neg_mask = target < THRESH_NEG
    pos_sum = float(np.sum(np.where(pos_mask, all_loss * weight, 0.0)))
    num_pos = int(np.sum(pos_mask))
    num_neg = int(np.sum(neg_mask))
    k = min(max(1000, 3 * num_pos), num_neg)
    neg_vals = all_loss[neg_mask]
    if k >= num_neg:
        topk = float(neg_vals.sum())
    elif k <= 0:
        topk = 0.0
    else:
        topk = float(np.partition(neg_vals, num_neg - k)[num_neg - k :].sum())
    return (pos_sum + topk) / (num_pos + k)


def kernel(output, character_map, affinity_map, character_weight, affinity_weight):
    from concourse.bass_utils import run_bass_kernel_spmd

    global LAST_RESULTS
    np_dt = ml_dtypes.bfloat16

    output = np.asarray(output, dtype=np.float32)

    def shard(a):
        # flat pixel order (b, h, w) -> [core, partition, free]
        return np.ascontiguousarray(a).reshape(N_CORES, P, FD).astype(np_dt)

    packed = np.empty((N_CORES, P, 2, 3, FD), dtype=np_dt)
    packed[:, :, 0, 0] = shard(output[:, 0])
    packed[:, :, 0, 1] = shard(np.asarray(character_map, dtype=np.float32))
    packed[:, :, 0, 2] = shard(np.asarray(character_weight, dtype=np.float32))
    packed[:, :, 1, 0] = shard(output[:, 1])
    packed[:, :, 1, 1] = shard(np.asarray(affinity_map, dtype=np.float32))
    packed[:, :, 1, 2] = shard(np.asarray(affinity_weight, dtype=np.float32))

    in_maps = [{"pk": packed[c]} for c in range(N_CORES)]

    nc = _get_nc()
    res = run_bass_kernel_spmd(
        nc,
        in_maps,
        list(range(N_CORES)),
        trace=os.environ.get("KERNEL_TRACE", "0") == "1",
    )
    LAST_RESULTS = res

    # [cores, PR, branch, group, col]
    acc_ps = np.stack([r["acc_ps"] for r in res.results]).astype(np.float64)
    idx = np.arange(BD)
    s1 = acc_ps[:, idx, :, 0, idx].sum(axis=(0, 1))       # [branch]
    s2 = acc_ps[:, idx, :, 2, idx].sum(axis=(0, 1))       # [branch]
    n_neg = acc_ps[:, BD, :, 0, :].sum(axis=(0, 2))       # [branch]
    n_pos = acc_ps[:, BD, :, 1, :].sum(axis=(0, 2))       # [branch]

    total = 0.0
    for bidx, (tmap, wmap) in enumerate(
        [(character_map, character_weight), (affinity_map, affinity_weight)]
    ):
        num_neg = int(round(n_neg[bidx]))
        num_pos = int(round(n_pos[bidx]))
        k = min(max(1000, 3 * num_pos), num_neg)
        if k == num_neg:
            total += (s1[bidx] + s2[bidx]) / (num_pos + k)
        else:
            # top-k actually selective: fall back to exact host computation
            total += _np_branch_fallback(
                output[:, bidx].reshape(-1),
                np.asarray(tmap, dtype=np.float32).reshape(-1),
                np.asarray(wmap, dtype=np.float32).reshape(-1),
            )

    return np.float32(total)


# revision 41
# speedup vs baseline: 1.1690x; 1.0054x over previous
"""Trainium2 Bass kernel for the CRAFT-style hard-negative-mining MSE loss.

Reference math (per branch, over N = 16*768*768 flat pixels):
    all_loss = (pred - target)^2
    pos_mask = (target >= 0.3) & (weight != 0)
    neg_mask = (target < 0.1)
    pos_sum  = sum(pos_mask * all_loss * weight)
    k        = min(max(1000, 3*num_pos), num_neg)
    topk_sum = sum of k largest all_loss among negatives
    loss     = (pos_sum + topk_sum) / (num_pos + k)
    out      = loss_char + loss_aff

With uniform targets num_pos ~ 0.7*N, so 3*num_pos >> num_neg and
k == num_neg: the top-k degenerates to the full sum over negatives.

Device strategy (v5, TensorE-assisted): per 1/8 shard, per branch-tile
[128, W]:
    DVE:  d  = p - t                    tensor_tensor        (2x mode)
          m_neg = (t < 0.1)            tensor_scalar is_lt  (4x mode)
          m_pos = (t >= 0.3)           tensor_scalar is_ge  (4x mode)
          mw = m_pos * w                tensor_tensor        (2x mode)
    ACT:  l  = d^2                      Square (1x) -> strided [nb, 97]
    PE:   nb = W/96 blocks of 96 data cols:
            psum[0:97, 0:288] += [l_96 | 1]^T @ [m_neg | m_pos | mw]
          accumulated over the whole branch into one [97, 288] PSUM
          region:
            diag(rows 0:96 of group 0) -> per-col <m_neg, l> = S1
            diag(rows 0:96 of group 2) -> per-col <mw,    l> = S2
            row 96 of group 0          -> per-col sums of m_neg = num_neg
            row 96 of group 1          -> per-col sums of m_pos = num_pos
The [97, 288] PSUM regions are copied to SBUF (ScalarE) and DMA'd out
in 4 partition slices over 4 DMA queues; the host extracts
diagonals/count-rows, sums across the 8 shards, and applies the
k/denominator logic (with a full numpy fallback for the never-hit-here
k < num_neg case).

Tile widths are uneven on purpose: DVE per-op fixed overhead (~0.3us
of init bubble + drain per instruction) favors wide ops, but wide ops
at the pipeline ends serialize against DMA.  So tiles are small at the
very start (quick pipeline fill) and very end (short drain tail), and
wide in the DMA-saturated middle.

This moves the masked-sum and count reductions off DVE/ACT (where they
only run at 1x) onto the otherwise-idle TensorE.  Inputs are cast to
bf16 on the host: halves HBM traffic and doubles DVE tensor_tensor
throughput.
"""

import os
import numpy as np
import ml_dtypes

N_CORES = 8
B, H, W = 16, 768, 768
NPX = B * H * W              # 9_437_184 flat pixels
P = 128                      # SBUF partitions
FD = NPX // (N_CORES * P)    # 9216 free-dim elements per core per tensor
BD = 96                      # data columns per matmul block
PR = BD + 1                  # psum rows used (96 data + 1 count row)
WMAX = 2304                  # widest tile

# per-branch tile widths: small tiles at the global start (pipeline
# fill) and global end (drain tail) only
WIDTHS = [
    [1152, 1152, 2304, 2304, 2304],   # branch 0
    [2304, 2304, 2304, 1152, 1152],   # branch 1
]

THRESH_NEG = 0.1
THRESH_POS = 0.3

_compiled = None             # cached nc
LAST_RESULTS = None          # BassKernelResults of the last run (for profiling)


def _build_nc():
    import concourse.bacc as bacc
    import concourse.mybir as mybir
    import concourse.tile as tile
    from contextlib import ExitStack

    DT = mybir.dt.bfloat16
    f32 = mybir.dt.float32
    Alu = mybir.AluOpType
    Act = mybir.ActivationFunctionType

    nc = bacc.Bacc(
        "TRN2",
        target_bir_lowering=False,
        debug=False,
        num_devices=N_CORES,
    )

    # packed input: [P, branch, (p,t,w), FD]
    pk = nc.declare_dram_parameter("pk", [P, 2, 3, FD], DT, isOutput=False)
    out_ps = nc.declare_dram_parameter("acc_ps", [PR, 2, 3, BD], f32, isOutput=True)

    with tile.TileContext(nc) as tc, ExitStack() as ctx:
        in_pool = ctx.enter_context(tc.tile_pool(name="in", bufs=4))
        d_pool = ctx.enter_context(tc.tile_pool(name="d", bufs=2))
        m_pool = ctx.enter_context(tc.tile_pool(name="m", bufs=3))
        acc_pool = ctx.enter_context(tc.tile_pool(name="acc", bufs=1))
        ps_pool = ctx.enter_context(tc.psum_pool(name="ps", bufs=1))

        ps_sb = acc_pool.tile([PR, 2, 3, BD], f32, tag="ps_sb")
        psum = [
            ps_pool.tile([PR, 3, BD], f32, tag=f"psum{b}", name=f"psum{b}")
            for b in range(2)
        ]
        # persistent double-buffered [l | ones] stationary tensors; the
        # ones column (col 96 of each 97-block) is written once up front
        # and never touched again
        NBMAX = WMAX // BD
        lexts = [
            acc_pool.tile([P, NBMAX, PR], DT, tag=f"lext{j}", name=f"lext{j}")
            for j in range(2)
        ]
        for j in range(2):
            nc.gpsimd.memset(lexts[j][:, 0:3, 0:PR], 0.0)
            nc.gpsimd.memset(lexts[j][:, :, BD : BD + 1], 1.0)

        # PE warm-up: the HAM clock gate keeps TensorE at 1.2 GHz until it
        # has been busy ~3.4us.  The first real matmul only lands ~15us in
        # (after DMA+DVE+ACT of tile 0), so burn the idle head on dummy
        # matmuls into a scratch PSUM bank so the PE is at 2.4 GHz when
        # the real stream starts.
        ps_warm = ps_pool.tile([PR, 3 * PR], f32, tag="ps_warm")
        for _ in range(34):
            nc.tensor.matmul(
                ps_warm[:, :],
                lexts[0][:, 0, :],
                lexts[0][:, 0:3, :],
                start=True,
                stop=True,
            )

        it = 0
        for b in range(2):
            c0 = 0
            for i, Wt in enumerate(WIDTHS[b]):
                nb = Wt // BD
                sl_in = slice(c0, c0 + Wt)
                tin = in_pool.tile([P, 3, WMAX], DT, tag="in")
                nc.sync.dma_start(tin[:, :, 0:Wt], pk[:, b, :, sl_in])
                pt = tin[:, 0, 0:Wt]
                tt = tin[:, 1, 0:Wt]
                wt = tin[:, 2, 0:Wt]

                # d = pred - target first, so ACT can start  (DVE TT 2x)
                d = d_pool.tile([P, WMAX], DT, tag="d")
                nc.vector.tensor_tensor(d[:, 0:Wt], pt, tt, Alu.subtract)
                # l = d^2 into cols 0:96 of the 97-blocks, overlapping the
                # mask ops below                             (ACT Square 1x)
                lext = lexts[it % 2]
                nc.scalar.activation(
                    lext[:, 0:nb, 0:BD], d[:, 0:Wt], Act.Square
                )
                # masks (DVE TS 4x):  m[:,0]=(t<0.1)  m[:,1]=(t>=0.3)
                m = m_pool.tile([P, 3, WMAX], DT, tag="m")
                nc.vector.tensor_scalar(
                    m[:, 0, 0:Wt], tt, THRESH_NEG, None, Alu.is_lt
                )
                nc.vector.tensor_scalar(
                    m[:, 1, 0:Wt], tt, THRESH_POS, None, Alu.is_ge
                )
                # m[:,2] = m_pos * w                         (DVE TT 2x)
                nc.vector.tensor_tensor(
                    m[:, 2, 0:Wt], m[:, 1, 0:Wt], wt, Alu.mult
                )

                # psum[b] += [l_blk | 1]^T @ [m_neg | m_pos | mw]   (PE)
                for k in range(nb):
                    sl = slice(k * BD, (k + 1) * BD)
                    nc.tensor.matmul(
                        psum[b][:, :, :],
                        lext[:, k, :],
                        m[:, :, sl],
                        start=(i == 0 and k == 0),
                        stop=(i == len(WIDTHS[b]) - 1 and k == nb - 1),
                    )
                c0 += Wt
                it += 1

            # dump the accumulated [97, 288] PSUM region to SBUF (ScalarE),
            # then DMA it out in 4 partition slices on 4 different engine
            # queues (a single contiguous store serializes on one DMA ring)
            nc.scalar.copy(ps_sb[:, b], psum[b][:, :, :])
            slices = [(0, 25), (25, 49), (49, 73), (73, PR)]
            issuers = [nc.sync, nc.gpsimd, nc.scalar, nc.gpsimd]
            for (p0, p1), eng in zip(slices, issuers):
                eng.dma_start(out_ps[p0:p1, b], ps_sb[p0:p1, b])

    nc.compile()
    return nc


def _get_nc():
    global _compiled
    if _compiled is None:
        _compiled = _build_nc()
    return _compiled


def _np_branch_fallback(pred, target, weight):
    """Exact reference math in numpy float64 (handles k < num_neg)."""
    pred = pred.astype(np.float64)
    target = target.astype(np.float64)
    weight = weight.astype(np.float64)
    all_loss = (pred - target) ** 2
    pos_mask = (target >= THRESH_POS) & (weight != 0)
    neg_mask = target < THRESH_NEG
    pos_sum = float(np.sum(np.where(pos_mask, all_loss * weight, 0.0)))
    num_pos = int(np.sum(pos_mask))
    num_neg = int(np.sum(neg_mask))
    k = min(max(1000, 3 * num_pos), num_neg)
    neg_vals = all_loss[neg_mask]
    if k >= num_neg:
        topk = float(neg_vals.sum())
    elif k <= 0:
        topk = 0.0
    else:
        topk = float(np.partition(neg_vals, num_neg - k)[num_neg - k :].sum())
    return (pos_sum + topk) / (num_pos + k)


def kernel(output, character_map, affinity_map, character_weight, affinity_weight):
    from concourse.bass_utils import run_bass_kernel_spmd

    global LAST_RESULTS
    np_dt = ml_dtypes.bfloat16

    output = np.asarray(output, dtype=np.float32)

    def shard(a):
        # flat pixel order (b, h, w) -> [core, partition, free]
        return np.ascontiguousarray(a).reshape(N_CORES, P, FD).astype(np_dt)

    packed = np.empty((N_CORES, P, 2, 3, FD), dtype=np_dt)
    packed[:, :, 0, 0] = shard(output[:, 0])
    packed[:, :, 0, 1] = shard(np.asarray(character_map, dtype=np.float32))
    packed[:, :, 0, 2] = shard(np.asarray(character_weight, dtype=np.float32))
    packed[:, :, 1, 0] = shard(output[:, 1])
    packed[:, :, 1, 1] = shard(np.asarray(affinity_map, dtype=np.float32))
    packed[:, :, 1, 2] = shard(np.asarray(affinity_weight, dtype=np.float32))

    in_maps = [{"pk": packed[c]} for c in range(N_CORES)]

    nc = _get_nc()
    res = run_bass_kernel_spmd(
        nc,
        in_maps,
        list(range(N_CORES)),
        trace=os.environ.get("KERNEL_TRACE", "0") == "1",
    )
    LAST_RESULTS = res

    # [cores, PR, branch, group, col]
    acc_ps = np.stack([r["acc_ps"] for r in res.results]).astype(np.float64)
    idx = np.arange(BD)
    s1 = acc_ps[:, idx, :, 0, idx].sum(axis=(0, 1))       # [branch]
    s2 = acc_ps[:, idx, :, 2, idx].sum(axis=(0, 1))       # [branch]
    n_neg = acc_ps[:, BD, :, 0, :].sum(axis=(0, 2))       # [branch]
    n_pos = acc_ps[:, BD, :, 1, :].sum(axis=(0, 2))       # [branch]

    total = 0.0
    for bidx, (tmap, wmap) in enumerate(
        [(character_map, character_weight), (affinity_map, affinity_weight)]
    ):
        num_neg = int(round(n_neg[bidx]))
        num_pos = int(round(n_pos[bidx]))
        k = min(max(1000, 3 * num_pos), num_neg)
        if k == num_neg:
            total += (s1[bidx] + s2[bidx]) / (num_pos + k)
        else:
            # top-k actually selective: fall back to exact host computation
            total += _np_branch_fallback(
                output[:, bidx].reshape(-1),
                np.asarray(tmap, dtype=np.float32).reshape(-1),
                np.asarray(wmap, dtype=np.float32).reshape(-1),
            )

    return np.float32(total)
